# revision 49
# baseline (speedup 1.0000x reference)
"""Trainium2 Bass kernel for nn_MultiHeadCrossAttention (8-core SPMD).

Sharding: core = (batch, head-half); data parallel over the 4 batches,
tensor parallel over the 16 heads (8 per core). All matmuls run in fp16
(PSUM accumulation stays fp32): 16-bit streams keep the PE HAM clock gate
at 8/8 without warm-up hacks. Scores use K=64 row tiling so the two heads
of a pair compute concurrently on the upper/lower halves of the PE array.
Exp is split between the Scalar engine (exact) and the Vector engine
(fp16 Schraudolph bit-trick, one tensor_scalar per slice). The o-projection
accumulates all four head-pair partials in PSUM on device.
"""
import sys

for p in ("/opt/trn_rl_repo", "/root/.axon_site/_ro/trn_rl_repo"):
    if p not in sys.path:
        sys.path.insert(0, p)


from contextlib import ExitStack

import concourse.bass as bass
import concourse.mybir as mybir
import concourse.tile as tile

F32 = mybir.dt.float32
FP16 = mybir.dt.float16
I16 = mybir.dt.int16
EXP = mybir.ActivationFunctionType.Exp
MULT = mybir.AluOpType.mult
ADD = mybir.AluOpType.add

DK = 64

# fp16 Schraudolph exp: bitcast_fp16(int16(round(S*x + B))) ~= exp(0.125*x)
EXP_S = 1024.0 * 1.4426950408889634 * 0.125
EXP_B = 15360.0 - 44.0
# every APPROX_PERIOD-th key chunk runs exp on the Vector engine via the
# bit-trick (~3% per weight); cutting along keys keeps any single query's
# softmax only 1/APPROX_PERIOD approximate
APPROX_PERIOD = 4


def declare_io(nc, S, F, H):
    HD = H * DK  # 512
    io = {}
    io["hsT"] = nc.dram_tensor("hsT", [F, S], FP16, kind="ExternalInput").ap()
    io["htT"] = nc.dram_tensor("htT", [F, S], FP16, kind="ExternalInput").ap()
    io["wqT"] = nc.dram_tensor("wqT", [F, HD], FP16, kind="ExternalInput").ap()
    io["wkT"] = nc.dram_tensor("wkT", [F, HD], FP16, kind="ExternalInput").ap()
    io["wvT"] = nc.dram_tensor("wvT", [F, HD], FP16, kind="ExternalInput").ap()
    io["woT"] = nc.dram_tensor("woT", [HD, F], FP16, kind="ExternalInput").ap()
    io["outT"] = nc.dram_tensor("outT", [F, S], F32, kind="ExternalOutput").ap()
    return io


def build(ctx: ExitStack, tc: tile.TileContext, io, S, F, H):
    nc = tc.nc
    HD = H * DK          # 512 qk/v features per core
    nF = F // 128        # 8 feature tiles
    nHP = H // 2         # 4 head pairs
    TBW = 512            # token block width (projections)
    nTB = S // TBW       # 4
    IBW = 512            # query block width (attention)
    nIB = S // IBW       # 4
    KC = 128             # keys per score chunk
    nJC = S // KC        # 16

    ec = ctx.enter_context
    ec(nc.allow_low_precision(reason="fp16 matmul pipeline; psum accum stays fp32"))
    consts = ec(tc.tile_pool(name="consts", bufs=1))
    hpool = ec(tc.tile_pool(name="hpool", bufs=1))      # hs+ht resident
    wpool = ec(tc.tile_pool(name="wpool", bufs=1))      # weights resident
    vpool = ec(tc.tile_pool(name="vpool", bufs=1))      # v|1 tiles resident
    qkpool = ec(tc.tile_pool(name="qkpool", bufs=1))    # kt/qt double set
    vtpool = ec(tc.tile_pool(name="vtpool", bufs=1))    # valsT resident
    work = ec(tc.tile_pool(name="work", bufs=3))        # e tiles
    rpool = ec(tc.tile_pool(name="rpool", bufs=2))      # recip rows
    ostage = ec(tc.tile_pool(name="ostage", bufs=3))    # o-proj staging
    scps = ec(tc.tile_pool(name="scps", bufs=2, space="PSUM"))   # scores 4 banks
    pvps = ec(tc.tile_pool(name="pvps", bufs=1, space="PSUM"))   # vals 2 banks
    prps = ec(tc.tile_pool(name="prps", bufs=2, space="PSUM"))   # proj 2 banks

    ones32 = consts.tile([1, 128], F32, tag="ones32")
    nc.vector.memset(ones32[:], 1.0)

    # ---- resident loads (phase-A dependencies first, column-blocked so the
    # first v-projection token blocks only wait on their own columns) ----
    hs_sb, ht_sb = [], []
    wq_sb, wk_sb, wv_sb = [], [], []
    for f in range(nF):
        t = wpool.tile([128, HD], FP16, tag=f"wv{f}", name=f"wv{f}")
        nc.scalar.dma_start(out=t[:], in_=io["wvT"][f * 128 : (f + 1) * 128, :])
        wv_sb.append(t)
        ht_sb.append(hpool.tile([128, S], FP16, tag=f"ht{f}", name=f"ht{f}"))
        hs_sb.append(hpool.tile([128, S], FP16, tag=f"hs{f}", name=f"hs{f}"))
    for cb in range(4):
        cs = slice(cb * 512, (cb + 1) * 512)
        for f in range(nF):
            nc.sync.dma_start(
                out=ht_sb[f][:, cs], in_=io["htT"][f * 128 : (f + 1) * 128, cs]
            )
        for f in range(nF):
            nc.gpsimd.dma_start(
                out=hs_sb[f][:, cs], in_=io["hsT"][f * 128 : (f + 1) * 128, cs]
            )
    for f in range(nF):
        for nm, src, dst in (("wq", "wqT", wq_sb), ("wk", "wkT", wk_sb)):
            t2 = wpool.tile([128, HD], FP16, tag=f"{nm}{f}", name=f"{nm}{f}")
            nc.scalar.dma_start(out=t2[:], in_=io[src][f * 128 : (f + 1) * 128, :])
            dst.append(t2)
    wo_sb = []
    for hp in range(nHP):
        t = wpool.tile([128, F], FP16, tag=f"wo{hp}", name=f"wo{hp}")
        nc.scalar.dma_start(out=t[:], in_=io["woT"][hp * 128 : (hp + 1) * 128, :])
        wo_sb.append(t)

    # ---- Phase A: v projection into [v|1]-interleaved token-major tiles ----
    v_sb = []
    for tb in range(S // 128):
        vt = vpool.tile([128, H * 65], FP16, tag=f"v{tb}", name=f"v{tb}")
        v_sb.append(vt)

    for tb in range(S // 128):
        nc.vector.memset(
            v_sb[tb][:].rearrange("p (h c) -> p h c", c=65)[:, :, 64:65], 1.0
        )

    def a_unit(tb):
        pa = prps.tile([128, TBW], F32, tag="pr", name="pa")[:, 0:HD]
        for f in range(nF):
            nc.tensor.matmul(
                pa[:],
                ht_sb[f][:, tb * 128 : (tb + 1) * 128],
                wv_sb[f][:, 0:HD],
                start=(f == 0),
                stop=(f == nF - 1),
            )
        nc.scalar.copy(
            v_sb[tb][:].rearrange("p (h c) -> p h c", c=65)[:, :, 0:64],
            pa[:].rearrange("p (h c) -> p h c", c=64),
        )

    # ---- B(hp): q/k projection units (resident weights + activations) ----
    kq_sets = []
    for par in range(2):
        kt = qkpool.tile([128, S], FP16, tag=f"kt{par}", name=f"kt{par}")
        qt = qkpool.tile([128, S], FP16, tag=f"qt{par}", name=f"qt{par}")
        kq_sets.append((kt, qt))

    def b_units(hp):
        kt, qt = kq_sets[hp % 2]
        units = []
        for tb in range(nTB):
            for which in ("q", "k"):

                def unit(tb=tb, which=which, hp=hp, kt=kt, qt=qt):
                    w_sb = wq_sb if which == "q" else wk_sb
                    dst = qt if which == "q" else kt
                    pq = prps.tile([128, TBW], F32, tag="pr", name="pq")
                    for f in range(nF):
                        nc.tensor.matmul(
                            pq[:],
                            w_sb[f][:, hp * 128 : (hp + 1) * 128],
                            hs_sb[f][:, tb * TBW : (tb + 1) * TBW],
                            start=(f == 0),
                            stop=(f == nF - 1),
                        )
                    nc.vector.tensor_copy(dst[:, tb * TBW : (tb + 1) * TBW], pq[:])

                units.append(unit)
        return kt, qt, units

    # interleave phase A with hp0's q/k projections for weight-load hiding
    kt_cur, qt_cur, bu = b_units(0)
    bu = list(bu)
    for tb in range(S // 128):
        a_unit(tb)
        if tb % 2 == 1 and bu:
            bu.pop(0)()
    while bu:
        bu.pop(0)()

    # ---- Phase C: attention with interleaved next-pair projections ----
    valsT_sb = [
        vtpool.tile([128, S], FP16, tag=f"vt{hp}", name=f"vt{hp}") for hp in range(nHP)
    ]

    queue = []    # deferred projection units for the next head pair
    pending = []  # deferred normalization closures

    def emit_av(e, jc, hp, pv_pair):
        for par in range(2):
            nc.tensor.matmul(
                pv_pair[par][:],
                v_sb[jc][:, (2 * hp + par) * 65 : (2 * hp + par + 1) * 65],
                e[:, par * 512 : (par + 1) * 512],
                start=(jc == 0),
                stop=(jc == nJC - 1),
            )

    post = []     # deferred pv -> SBUF drains (run early next query block)

    def make_post(hp, ibs, pv_pair, par):
        h_rows = slice(par * 64, (par + 1) * 64)

        def drain():
            d = rpool.tile([1, IBW], F32, tag="d", name="d")
            nc.vector.tensor_copy(d[:], pv_pair[par][64:65, :])
            r = rpool.tile([1, IBW], F32, tag="r", name="r", bufs=4)
            nc.vector.reciprocal_approx_fast(out=r[:], in_=d[:])
            nc.vector.tensor_copy(valsT_sb[hp][h_rows, ibs], pv_pair[par][0:64, :])

            def norm():
                pb = prps.tile([128, TBW], F32, tag="pr", name="pb")[:, 0:IBW]
                nc.tensor.matmul(pb[:], ones32[:], r[:], start=True, stop=True)
                nc.vector.tensor_mul(
                    valsT_sb[hp][h_rows, ibs],
                    valsT_sb[hp][h_rows, ibs],
                    pb[h_rows, :],
                )

            pending.append(norm)

        return drain

    for hp in range(nHP):
        nxt = b_units(hp + 1) if hp + 1 < nHP else None
        if nxt:
            queue.extend(nxt[2])

        for ib in range(nIB):
            ibs = slice(ib * IBW, (ib + 1) * IBW)
            pv_pair = [
                pvps.tile([128, IBW], F32, tag=f"pv{par}", name=f"pv{par}")[0:65, :]
                for par in range(2)
            ]
            pend = []
            for jc in range(nJC):
                sc = scps.tile([128, 1024], F32, tag="sc", name="sc")
                for par in range(2):
                    nc.tensor.matmul(
                        sc[:, par * 512 : (par + 1) * 512],
                        kt_cur[par * 64 : (par + 1) * 64, jc * 128 : (jc + 1) * 128],
                        qt_cur[par * 64 : (par + 1) * 64, ibs],
                        start=True,
                        stop=True,
                    )
                if len(pend) >= 2:
                    emit_av(*pend.pop(0), hp, pv_pair)
                # slot schedule: pack PE-heavy fillers into the pipeline-fill
                # phase (jc 0-3) so the HAM activity window never sees a lull
                if jc in (0, 3):
                    if queue:
                        queue.pop(0)()
                    else:
                        # HAM-warm filler on resident data (dependency-free so
                        # it can never head-of-line-block the PE); discarded
                        pw = prps.tile([128, TBW], F32, tag="pr", name="pw")
                        for _ in range(3):
                            nc.tensor.matmul(
                                pw[:], kt_cur[0:64, 0:128], qt_cur[0:64, 0:512],
                                start=True, stop=True,
                            )
                elif jc in (1, 2) and post:
                    post.pop(0)()
                elif jc in (5, 7) and pending:
                    pending.pop(0)()
                elif jc in (8, 11) and not queue:
                    pw = prps.tile([128, TBW], F32, tag="pr", name="pw")
                    for _ in range(2):
                        nc.tensor.matmul(
                            pw[:], kt_cur[0:64, 0:128], qt_cur[0:64, 0:512],
                            start=True, stop=True,
                        )
                e = work.tile([128, 1024], FP16, tag="e")
                if jc % APPROX_PERIOD == 0:
                    nc.vector.tensor_scalar(
                        e[:].bitcast(I16)[:], sc[:], EXP_S, EXP_B, MULT, ADD
                    )
                else:
                    nc.scalar.activation(e[:], sc[:], EXP, scale=0.125)
                pend.append((e, jc))
            while pend:
                emit_av(*pend.pop(0), hp, pv_pair)
            for par in range(2):
                post.append(make_post(hp, ibs, pv_pair, par))
        if nxt:
            kt_cur, qt_cur = nxt[0], nxt[1]

    while queue:
        queue.pop(0)()
    while post:
        post.pop(0)()
    while pending:
        pending.pop(0)()

    # ---- Phase D: o-projection, PSUM-accumulated over head pairs.
    # tb pairs share each LDWEIGHTS of the stationary wo chunk.
    for mb in range(nF):
        for tbp in range(nTB // 2):
            pos = [prps.tile([128, TBW], F32, tag="pr", name=f"po{t}") for t in range(2)]
            for hp in range(nHP):
                for t in range(2):
                    nc.tensor.matmul(
                        pos[t][:],
                        wo_sb[hp][:, mb * 128 : (mb + 1) * 128],
                        valsT_sb[hp][:, (2 * tbp + t) * TBW : (2 * tbp + t + 1) * TBW],
                        start=(hp == 0),
                        stop=(hp == nHP - 1),
                    )
            for t in range(2):
                tb = 2 * tbp + t
                ot = ostage.tile([128, TBW], F32, tag="ot")
                nc.scalar.copy(ot[:], pos[t][:])
                nc.sync.dma_start(
                    out=io["outT"][mb * 128 : (mb + 1) * 128, tb * TBW : (tb + 1) * TBW],
                    in_=ot[:],
                )


# ---- host orchestration ----


import numpy as np

N_CORES = 8
B_FULL, S_FULL, F_FULL = 4, 2048, 1024
H_TOTAL = 16
H_PER_CORE = H_TOTAL // 2

_compiled = {}


def _get_compiled():
    if "nc" not in _compiled:
        from contextlib import ExitStack

        from concourse import bacc

        nc = bacc.Bacc(
            "TRN2", target_bir_lowering=False, debug=False, num_devices=N_CORES
        )
        io = declare_io(nc, S_FULL, F_FULL, H_PER_CORE)
        with tile.TileContext(nc) as tc:
            with ExitStack() as ctx:
                build(ctx, tc, io, S_FULL, F_FULL, H_PER_CORE)
        nc.compile()
        _compiled["nc"] = nc
    return _compiled["nc"]


def _shard_inputs(h_source, h_target, w_qk, w_v, w_o):
    """Per-core input maps. Core c -> batch c//2, head-half c%2."""

    def c16(x):
        return np.ascontiguousarray(x.astype(np.float16))

    in_maps = []
    for core in range(N_CORES):
        b, hh = divmod(core, 2)
        heads = range(hh * H_PER_CORE, (hh + 1) * H_PER_CORE)
        wq = np.concatenate([w_qk[h * 128 : h * 128 + 64] for h in heads], 0)
        wk = np.concatenate([w_qk[h * 128 + 64 : (h + 1) * 128] for h in heads], 0)
        wv = np.concatenate([w_v[h * 64 : (h + 1) * 64] for h in heads], 0)
        dcols = np.concatenate([np.arange(h * 64, (h + 1) * 64) for h in heads])
        in_maps.append(
            {
                "hsT": c16(h_source[b].T),
                "htT": c16(h_target[b].T),
                "wqT": c16(wq.T),
                "wkT": c16(wk.T),
                "wvT": c16(wv.T),
                "woT": c16(w_o[:, dcols].T),
            }
        )
    return in_maps


def _run(h_source, h_target, w_qk, w_v, w_o, b_o, trace=False, trace_cores=None):
    from concourse.bass_utils import run_bass_kernel_spmd

    nc = _get_compiled()
    in_maps = _shard_inputs(
        np.asarray(h_source, np.float32),
        np.asarray(h_target, np.float32),
        np.asarray(w_qk, np.float32),
        np.asarray(w_v, np.float32),
        np.asarray(w_o, np.float32),
    )
    res = run_bass_kernel_spmd(
        nc,
        in_maps,
        core_ids=list(range(N_CORES)),
        trace=trace,
        trace_cores=trace_cores,
    )
    b_o = np.asarray(b_o, np.float32)
    out = np.empty((B_FULL, S_FULL, F_FULL), np.float32)
    for b in range(B_FULL):
        acc = res.results[2 * b]["outT"] + res.results[2 * b + 1]["outT"]
        out[b] = acc.T + b_o
    return out, res


def kernel(h_source, h_target, w_qk, w_v, w_o, b_o):
    out, _ = _run(h_source, h_target, w_qk, w_v, w_o, b_o)
    return out


# revision 51
# speedup vs baseline: 1.0171x; 1.0171x over previous
"""Trainium2 Bass kernel for nn_MultiHeadCrossAttention (8-core SPMD).

Sharding: core = (batch, head-half); data parallel over the 4 batches,
tensor parallel over the 16 heads (8 per core). All matmuls run in fp16
(PSUM accumulation stays fp32): 16-bit streams keep the PE HAM clock gate
at 8/8 without warm-up hacks. Scores use K=64 row tiling so the two heads
of a pair compute concurrently on the upper/lower halves of the PE array.
Exp is split between the Scalar engine (exact) and the Vector engine
(fp16 Schraudolph bit-trick, one tensor_scalar per slice). The o-projection
accumulates all four head-pair partials in PSUM on device.
"""
import sys

for p in ("/opt/trn_rl_repo", "/root/.axon_site/_ro/trn_rl_repo"):
    if p not in sys.path:
        sys.path.insert(0, p)


from contextlib import ExitStack

import concourse.bass as bass
import concourse.mybir as mybir
import concourse.tile as tile

F32 = mybir.dt.float32
FP16 = mybir.dt.float16
I16 = mybir.dt.int16
EXP = mybir.ActivationFunctionType.Exp
MULT = mybir.AluOpType.mult
ADD = mybir.AluOpType.add

DK = 64

# fp16 Schraudolph exp: bitcast_fp16(int16(round(S*x + B))) ~= exp(0.125*x)
EXP_S = 1024.0 * 1.4426950408889634 * 0.125
EXP_B = 15360.0 - 44.0
# every APPROX_PERIOD-th key chunk runs exp on the Vector engine via the
# bit-trick (~3% per weight); cutting along keys keeps any single query's
# softmax only 1/APPROX_PERIOD approximate
APPROX_PERIOD = 4


def declare_io(nc, S, F, H):
    HD = H * DK  # 512
    io = {}
    io["hsT"] = nc.dram_tensor("hsT", [F, S], FP16, kind="ExternalInput").ap()
    io["htT"] = nc.dram_tensor("htT", [F, S], FP16, kind="ExternalInput").ap()
    io["wqT"] = nc.dram_tensor("wqT", [F, HD], FP16, kind="ExternalInput").ap()
    io["wkT"] = nc.dram_tensor("wkT", [F, HD], FP16, kind="ExternalInput").ap()
    io["wvT"] = nc.dram_tensor("wvT", [F, HD], FP16, kind="ExternalInput").ap()
    io["woT"] = nc.dram_tensor("woT", [HD, F], FP16, kind="ExternalInput").ap()
    io["outT"] = nc.dram_tensor("outT", [F, S], F32, kind="ExternalOutput").ap()
    return io


def build(ctx: ExitStack, tc: tile.TileContext, io, S, F, H):
    nc = tc.nc
    HD = H * DK          # 512 qk/v features per core
    nF = F // 128        # 8 feature tiles
    nHP = H // 2         # 4 head pairs
    TBW = 512            # token block width (projections)
    nTB = S // TBW       # 4
    IBW = 512            # query block width (attention)
    nIB = S // IBW       # 4
    KC = 128             # keys per score chunk
    nJC = S // KC        # 16

    ec = ctx.enter_context
    ec(nc.allow_low_precision(reason="fp16 matmul pipeline; psum accum stays fp32"))
    consts = ec(tc.tile_pool(name="consts", bufs=1))
    hpool = ec(tc.tile_pool(name="hpool", bufs=1))      # hs+ht resident
    wpool = ec(tc.tile_pool(name="wpool", bufs=1))      # weights resident
    vpool = ec(tc.tile_pool(name="vpool", bufs=1))      # v|1 tiles resident
    qkpool = ec(tc.tile_pool(name="qkpool", bufs=1))    # kt/qt double set
    vtpool = ec(tc.tile_pool(name="vtpool", bufs=1))    # valsT resident
    work = ec(tc.tile_pool(name="work", bufs=3))        # e tiles
    rpool = ec(tc.tile_pool(name="rpool", bufs=2))      # recip rows
    ostage = ec(tc.tile_pool(name="ostage", bufs=3))    # o-proj staging
    scps = ec(tc.tile_pool(name="scps", bufs=2, space="PSUM"))   # scores 4 banks
    pvps = ec(tc.tile_pool(name="pvps", bufs=1, space="PSUM"))   # vals 2 banks
    prps = ec(tc.tile_pool(name="prps", bufs=2, space="PSUM"))   # proj 2 banks

    ones32 = consts.tile([1, 128], F32, tag="ones32")
    nc.vector.memset(ones32[:], 1.0)

    # ---- resident loads (phase-A dependencies first, column-blocked so the
    # first v-projection token blocks only wait on their own columns) ----
    hs_sb, ht_sb = [], []
    wq_sb, wk_sb, wv_sb = [], [], []
    for f in range(nF):
        t = wpool.tile([128, HD], FP16, tag=f"wv{f}", name=f"wv{f}")
        nc.scalar.dma_start(out=t[:], in_=io["wvT"][f * 128 : (f + 1) * 128, :])
        wv_sb.append(t)
        ht_sb.append(hpool.tile([128, S], FP16, tag=f"ht{f}", name=f"ht{f}"))
        hs_sb.append(hpool.tile([128, S], FP16, tag=f"hs{f}", name=f"hs{f}"))
    for cb in range(4):
        cs = slice(cb * 512, (cb + 1) * 512)
        for f in range(nF):
            nc.sync.dma_start(
                out=ht_sb[f][:, cs], in_=io["htT"][f * 128 : (f + 1) * 128, cs]
            )
        for f in range(nF):
            nc.sync.dma_start(
                out=hs_sb[f][:, cs], in_=io["hsT"][f * 128 : (f + 1) * 128, cs]
            )
    for f in range(nF):
        for nm, src, dst in (("wq", "wqT", wq_sb), ("wk", "wkT", wk_sb)):
            t2 = wpool.tile([128, HD], FP16, tag=f"{nm}{f}", name=f"{nm}{f}")
            nc.scalar.dma_start(out=t2[:], in_=io[src][f * 128 : (f + 1) * 128, :])
            dst.append(t2)
    wo_sb = []
    for hp in range(nHP):
        t = wpool.tile([128, F], FP16, tag=f"wo{hp}", name=f"wo{hp}")
        nc.scalar.dma_start(out=t[:], in_=io["woT"][hp * 128 : (hp + 1) * 128, :])
        wo_sb.append(t)

    # ---- Phase A: v projection into [v|1]-interleaved token-major tiles ----
    v_sb = []
    for tb in range(S // 128):
        vt = vpool.tile([128, H * 65], FP16, tag=f"v{tb}", name=f"v{tb}")
        v_sb.append(vt)

    for tb in range(S // 128):
        nc.vector.memset(
            v_sb[tb][:].rearrange("p (h c) -> p h c", c=65)[:, :, 64:65], 1.0
        )

    def a_unit(tb):
        pa = prps.tile([128, TBW], F32, tag="pr", name="pa")[:, 0:HD]
        for f in range(nF):
            nc.tensor.matmul(
                pa[:],
                ht_sb[f][:, tb * 128 : (tb + 1) * 128],
                wv_sb[f][:, 0:HD],
                start=(f == 0),
                stop=(f == nF - 1),
            )
        nc.scalar.copy(
            v_sb[tb][:].rearrange("p (h c) -> p h c", c=65)[:, :, 0:64],
            pa[:].rearrange("p (h c) -> p h c", c=64),
        )

    # ---- B(hp): q/k projection units (resident weights + activations) ----
    kq_sets = []
    for par in range(2):
        kt = qkpool.tile([128, S], FP16, tag=f"kt{par}", name=f"kt{par}")
        qt = qkpool.tile([128, S], FP16, tag=f"qt{par}", name=f"qt{par}")
        kq_sets.append((kt, qt))

    def b_units(hp):
        kt, qt = kq_sets[hp % 2]
        units = []
        for tb in range(nTB):
            for which in ("q", "k"):

                def unit(tb=tb, which=which, hp=hp, kt=kt, qt=qt):
                    w_sb = wq_sb if which == "q" else wk_sb
                    dst = qt if which == "q" else kt
                    pq = prps.tile([128, TBW], F32, tag="pr", name="pq")
                    for f in range(nF):
                        nc.tensor.matmul(
                            pq[:],
                            w_sb[f][:, hp * 128 : (hp + 1) * 128],
                            hs_sb[f][:, tb * TBW : (tb + 1) * TBW],
                            start=(f == 0),
                            stop=(f == nF - 1),
                        )
                    nc.vector.tensor_copy(dst[:, tb * TBW : (tb + 1) * TBW], pq[:])

                units.append(unit)
        return kt, qt, units

    # interleave phase A with hp0's q/k projections for weight-load hiding
    kt_cur, qt_cur, bu = b_units(0)
    bu = list(bu)
    for tb in range(S // 128):
        a_unit(tb)
        if tb % 2 == 1 and bu:
            bu.pop(0)()
    while bu:
        bu.pop(0)()

    # ---- Phase C: attention with interleaved next-pair projections ----
    valsT_sb = [
        vtpool.tile([128, S], FP16, tag=f"vt{hp}", name=f"vt{hp}") for hp in range(nHP)
    ]

    queue = []    # deferred projection units for the next head pair
    pending = []  # deferred normalization closures

    def emit_av(e, jc, hp, pv_pair):
        for par in range(2):
            nc.tensor.matmul(
                pv_pair[par][:],
                v_sb[jc][:, (2 * hp + par) * 65 : (2 * hp + par + 1) * 65],
                e[:, par * 512 : (par + 1) * 512],
                start=(jc == 0),
                stop=(jc == nJC - 1),
            )

    post = []     # deferred pv -> SBUF drains (run early next query block)

    def make_post(hp, ibs, pv_pair, par):
        h_rows = slice(par * 64, (par + 1) * 64)

        def drain():
            d = rpool.tile([1, IBW], F32, tag="d", name="d")
            nc.vector.tensor_copy(d[:], pv_pair[par][64:65, :])
            r = rpool.tile([1, IBW], F32, tag="r", name="r", bufs=4)
            nc.vector.reciprocal_approx_fast(out=r[:], in_=d[:])
            nc.vector.tensor_copy(valsT_sb[hp][h_rows, ibs], pv_pair[par][0:64, :])

            def norm():
                pb = prps.tile([128, TBW], F32, tag="pr", name="pb")[:, 0:IBW]
                nc.tensor.matmul(pb[:], ones32[:], r[:], start=True, stop=True)
                nc.vector.tensor_mul(
                    valsT_sb[hp][h_rows, ibs],
                    valsT_sb[hp][h_rows, ibs],
                    pb[h_rows, :],
                )

            pending.append(norm)

        return drain

    for hp in range(nHP):
        nxt = b_units(hp + 1) if hp + 1 < nHP else None
        if nxt:
            queue.extend(nxt[2])

        for ib in range(nIB):
            ibs = slice(ib * IBW, (ib + 1) * IBW)
            pv_pair = [
                pvps.tile([128, IBW], F32, tag=f"pv{par}", name=f"pv{par}")[0:65, :]
                for par in range(2)
            ]
            pend = []
            for jc in range(nJC):
                sc = scps.tile([128, 1024], F32, tag="sc", name="sc")
                for par in range(2):
                    nc.tensor.matmul(
                        sc[:, par * 512 : (par + 1) * 512],
                        kt_cur[par * 64 : (par + 1) * 64, jc * 128 : (jc + 1) * 128],
                        qt_cur[par * 64 : (par + 1) * 64, ibs],
                        start=True,
                        stop=True,
                    )
                if len(pend) >= 2:
                    emit_av(*pend.pop(0), hp, pv_pair)
                # slot schedule: pack PE-heavy fillers into the pipeline-fill
                # phase (jc 0-3) so the HAM activity window never sees a lull
                if jc in (0, 3):
                    if queue:
                        queue.pop(0)()
                    else:
                        # HAM-warm filler on resident data (dependency-free so
                        # it can never head-of-line-block the PE); discarded
                        pw = prps.tile([128, TBW], F32, tag="pr", name="pw")
                        for _ in range(3):
                            nc.tensor.matmul(
                                pw[:], kt_cur[0:64, 0:128], qt_cur[0:64, 0:512],
                                start=True, stop=True,
                            )
                elif jc in (1, 2) and post:
                    post.pop(0)()
                elif jc in (5, 7) and pending:
                    pending.pop(0)()
                e = work.tile([128, 1024], FP16, tag="e")
                if jc % APPROX_PERIOD == 0:
                    nc.vector.tensor_scalar(
                        e[:].bitcast(I16)[:], sc[:], EXP_S, EXP_B, MULT, ADD
                    )
                else:
                    nc.scalar.activation(e[:], sc[:], EXP, scale=0.125)
                pend.append((e, jc))
            while pend:
                emit_av(*pend.pop(0), hp, pv_pair)
            for par in range(2):
                post.append(make_post(hp, ibs, pv_pair, par))
        if nxt:
            kt_cur, qt_cur = nxt[0], nxt[1]

    while queue:
        queue.pop(0)()
    while post:
        post.pop(0)()
    while pending:
        pending.pop(0)()

    # ---- Phase D: o-projection, PSUM-accumulated over head pairs.
    # tb pairs share each LDWEIGHTS of the stationary wo chunk.
    for mb in range(nF):
        for tbp in range(nTB // 2):
            pos = [prps.tile([128, TBW], F32, tag="pr", name=f"po{t}") for t in range(2)]
            for hp in range(nHP):
                for t in range(2):
                    nc.tensor.matmul(
                        pos[t][:],
                        wo_sb[hp][:, mb * 128 : (mb + 1) * 128],
                        valsT_sb[hp][:, (2 * tbp + t) * TBW : (2 * tbp + t + 1) * TBW],
                        start=(hp == 0),
                        stop=(hp == nHP - 1),
                    )
            for t in range(2):
                tb = 2 * tbp + t
                ot = ostage.tile([128, TBW], F32, tag="ot")
                nc.scalar.copy(ot[:], pos[t][:])
                nc.sync.dma_start(
                    out=io["outT"][mb * 128 : (mb + 1) * 128, tb * TBW : (tb + 1) * TBW],
                    in_=ot[:],
                )


# ---- host orchestration ----


import numpy as np

N_CORES = 8
B_FULL, S_FULL, F_FULL = 4, 2048, 1024
H_TOTAL = 16
H_PER_CORE = H_TOTAL // 2

_compiled = {}


def _get_compiled():
    if "nc" not in _compiled:
        from contextlib import ExitStack

        from concourse import bacc

        nc = bacc.Bacc(
            "TRN2", target_bir_lowering=False, debug=False, num_devices=N_CORES
        )
        io = declare_io(nc, S_FULL, F_FULL, H_PER_CORE)
        with tile.TileContext(nc) as tc:
            with ExitStack() as ctx:
                build(ctx, tc, io, S_FULL, F_FULL, H_PER_CORE)
        nc.compile()
        _compiled["nc"] = nc
    return _compiled["nc"]


def _shard_inputs(h_source, h_target, w_qk, w_v, w_o):
    """Per-core input maps. Core c -> batch c//2, head-half c%2."""

    def c16(x):
        return np.ascontiguousarray(x.astype(np.float16))

    in_maps = []
    for core in range(N_CORES):
        b, hh = divmod(core, 2)
        heads = range(hh * H_PER_CORE, (hh + 1) * H_PER_CORE)
        wq = np.concatenate([w_qk[h * 128 : h * 128 + 64] for h in heads], 0)
        wk = np.concatenate([w_qk[h * 128 + 64 : (h + 1) * 128] for h in heads], 0)
        wv = np.concatenate([w_v[h * 64 : (h + 1) * 64] for h in heads], 0)
        dcols = np.concatenate([np.arange(h * 64, (h + 1) * 64) for h in heads])
        in_maps.append(
            {
                "hsT": c16(h_source[b].T),
                "htT": c16(h_target[b].T),
                "wqT": c16(wq.T),
                "wkT": c16(wk.T),
                "wvT": c16(wv.T),
                "woT": c16(w_o[:, dcols].T),
            }
        )
    return in_maps


def _run(h_source, h_target, w_qk, w_v, w_o, b_o, trace=False, trace_cores=None):
    from concourse.bass_utils import run_bass_kernel_spmd

    nc = _get_compiled()
    in_maps = _shard_inputs(
        np.asarray(h_source, np.float32),
        np.asarray(h_target, np.float32),
        np.asarray(w_qk, np.float32),
        np.asarray(w_v, np.float32),
        np.asarray(w_o, np.float32),
    )
    res = run_bass_kernel_spmd(
        nc,
        in_maps,
        core_ids=list(range(N_CORES)),
        trace=trace,
        trace_cores=trace_cores,
    )
    b_o = np.asarray(b_o, np.float32)
    out = np.empty((B_FULL, S_FULL, F_FULL), np.float32)
    for b in range(B_FULL):
        acc = res.results[2 * b]["outT"] + res.results[2 * b + 1]["outT"]
        out[b] = acc.T + b_o
    return out, res


def kernel(h_source, h_target, w_qk, w_v, w_o, b_o):
    out, _ = _run(h_source, h_target, w_qk, w_v, w_o, b_o)
    return out


# revision 55
# speedup vs baseline: 1.0203x; 1.0032x over previous
"""Trainium2 Bass kernel for nn_MultiHeadCrossAttention (8-core SPMD).

Sharding: core = (batch, head-half); data parallel over the 4 batches,
tensor parallel over the 16 heads (8 per core). All matmuls run in fp16
(PSUM accumulation stays fp32): 16-bit streams keep the PE HAM clock gate
at 8/8 without warm-up hacks. Scores use K=64 row tiling so the two heads
of a pair compute concurrently on the upper/lower halves of the PE array.
Exp is split between the Scalar engine (exact) and the Vector engine
(fp16 Schraudolph bit-trick, one tensor_scalar per slice). The o-projection
accumulates all four head-pair partials in PSUM on device.
"""
import sys

for p in ("/opt/trn_rl_repo", "/root/.axon_site/_ro/trn_rl_repo"):
    if p not in sys.path:
        sys.path.insert(0, p)


from contextlib import ExitStack

import concourse.bass as bass
import concourse.mybir as mybir
import concourse.tile as tile

F32 = mybir.dt.float32
FP16 = mybir.dt.float16
I16 = mybir.dt.int16
EXP = mybir.ActivationFunctionType.Exp
MULT = mybir.AluOpType.mult
ADD = mybir.AluOpType.add

DK = 64

# fp16 Schraudolph exp: bitcast_fp16(int16(round(S*x + B))) ~= exp(0.125*x)
EXP_S = 1024.0 * 1.4426950408889634 * 0.125
EXP_B = 15360.0 - 44.0
# every APPROX_PERIOD-th key chunk runs exp on the Vector engine via the
# bit-trick (~3% per weight); cutting along keys keeps any single query's
# softmax only 1/APPROX_PERIOD approximate
APPROX_PERIOD = 4


def declare_io(nc, S, F, H):
    HD = H * DK  # 512
    io = {}
    io["hsT"] = nc.dram_tensor("hsT", [F, S], FP16, kind="ExternalInput").ap()
    io["htT"] = nc.dram_tensor("htT", [F, S], FP16, kind="ExternalInput").ap()
    io["wqT"] = nc.dram_tensor("wqT", [F, HD], FP16, kind="ExternalInput").ap()
    io["wkT"] = nc.dram_tensor("wkT", [F, HD], FP16, kind="ExternalInput").ap()
    io["wvT"] = nc.dram_tensor("wvT", [F, HD], FP16, kind="ExternalInput").ap()
    io["woT"] = nc.dram_tensor("woT", [HD, F], FP16, kind="ExternalInput").ap()
    io["outT"] = nc.dram_tensor("outT", [F, S], F32, kind="ExternalOutput").ap()
    return io


def build(ctx: ExitStack, tc: tile.TileContext, io, S, F, H):
    nc = tc.nc
    HD = H * DK          # 512 qk/v features per core
    nF = F // 128        # 8 feature tiles
    nHP = H // 2         # 4 head pairs
    TBW = 512            # token block width (projections)
    nTB = S // TBW       # 4
    IBW = 512            # query block width (attention)
    nIB = S // IBW       # 4
    KC = 128             # keys per score chunk
    nJC = S // KC        # 16

    ec = ctx.enter_context
    ec(nc.allow_low_precision(reason="fp16 matmul pipeline; psum accum stays fp32"))
    consts = ec(tc.tile_pool(name="consts", bufs=1))
    hpool = ec(tc.tile_pool(name="hpool", bufs=1))      # hs+ht resident
    wpool = ec(tc.tile_pool(name="wpool", bufs=1))      # weights resident
    vpool = ec(tc.tile_pool(name="vpool", bufs=1))      # v|1 tiles resident
    qkpool = ec(tc.tile_pool(name="qkpool", bufs=1))    # kt/qt double set
    vtpool = ec(tc.tile_pool(name="vtpool", bufs=1))    # valsT resident
    work = ec(tc.tile_pool(name="work", bufs=3))        # e tiles
    rpool = ec(tc.tile_pool(name="rpool", bufs=2))      # recip rows
    ostage = ec(tc.tile_pool(name="ostage", bufs=3))    # o-proj staging
    scps = ec(tc.tile_pool(name="scps", bufs=2, space="PSUM"))   # scores 4 banks
    pvps = ec(tc.tile_pool(name="pvps", bufs=1, space="PSUM"))   # vals 2 banks
    prps = ec(tc.tile_pool(name="prps", bufs=2, space="PSUM"))   # proj 2 banks

    ones32 = consts.tile([1, 128], F32, tag="ones32")
    nc.vector.memset(ones32[:], 1.0)

    # ---- resident loads (phase-A dependencies first, column-blocked so the
    # first v-projection token blocks only wait on their own columns) ----
    hs_sb, ht_sb = [], []
    wq_sb, wk_sb, wv_sb = [], [], []
    for f in range(nF):
        t = wpool.tile([128, HD], FP16, tag=f"wv{f}", name=f"wv{f}")
        nc.scalar.dma_start(out=t[:], in_=io["wvT"][f * 128 : (f + 1) * 128, :])
        wv_sb.append(t)
        ht_sb.append(hpool.tile([128, S], FP16, tag=f"ht{f}", name=f"ht{f}"))
        hs_sb.append(hpool.tile([128, S], FP16, tag=f"hs{f}", name=f"hs{f}"))
    for f in range(nF):
        nc.sync.dma_start(out=ht_sb[f][:], in_=io["htT"][f * 128 : (f + 1) * 128, :])
    for f in range(nF):
        nc.gpsimd.dma_start(out=hs_sb[f][:], in_=io["hsT"][f * 128 : (f + 1) * 128, :])
    for f in range(nF):
        for nm, src, dst in (("wq", "wqT", wq_sb), ("wk", "wkT", wk_sb)):
            t2 = wpool.tile([128, HD], FP16, tag=f"{nm}{f}", name=f"{nm}{f}")
            nc.scalar.dma_start(out=t2[:], in_=io[src][f * 128 : (f + 1) * 128, :])
            dst.append(t2)
    wo_sb = []
    for hp in range(nHP):
        t = wpool.tile([128, F], FP16, tag=f"wo{hp}", name=f"wo{hp}")
        nc.scalar.dma_start(out=t[:], in_=io["woT"][hp * 128 : (hp + 1) * 128, :])
        wo_sb.append(t)

    # ---- Phase A: v projection into [v|1]-interleaved token-major tiles ----
    v_sb = []
    for tb in range(S // 128):
        vt = vpool.tile([128, H * 65], FP16, tag=f"v{tb}", name=f"v{tb}")
        v_sb.append(vt)

    for tb in range(S // 128):
        nc.vector.memset(
            v_sb[tb][:].rearrange("p (h c) -> p h c", c=65)[:, :, 64:65], 1.0
        )

    def a_unit(tb):
        pa = prps.tile([128, TBW], F32, tag="pr", name="pa")[:, 0:HD]
        for f in range(nF):
            nc.tensor.matmul(
                pa[:],
                ht_sb[f][:, tb * 128 : (tb + 1) * 128],
                wv_sb[f][:, 0:HD],
                start=(f == 0),
                stop=(f == nF - 1),
            )
        nc.scalar.copy(
            v_sb[tb][:].rearrange("p (h c) -> p h c", c=65)[:, :, 0:64],
            pa[:].rearrange("p (h c) -> p h c", c=64),
        )

    # ---- B(hp): q/k projection units (resident weights + activations) ----
    kq_sets = []
    for par in range(2):
        kt = qkpool.tile([128, S], FP16, tag=f"kt{par}", name=f"kt{par}")
        qt = qkpool.tile([128, S], FP16, tag=f"qt{par}", name=f"qt{par}")
        kq_sets.append((kt, qt))

    def b_units(hp):
        # k-units first: attention block ib0 needs the FULL kt but only the
        # first query block, so later q-units can hide inside attention
        kt, qt = kq_sets[hp % 2]
        units = []
        for which in ("k", "q"):
            for tb in range(nTB):

                def unit(tb=tb, which=which, hp=hp, kt=kt, qt=qt):
                    w_sb = wq_sb if which == "q" else wk_sb
                    dst = qt if which == "q" else kt
                    pq = prps.tile([128, TBW], F32, tag="pr", name="pq")
                    for f in range(nF):
                        nc.tensor.matmul(
                            pq[:],
                            w_sb[f][:, hp * 128 : (hp + 1) * 128],
                            hs_sb[f][:, tb * TBW : (tb + 1) * TBW],
                            start=(f == 0),
                            stop=(f == nF - 1),
                        )
                    nc.vector.tensor_copy(dst[:, tb * TBW : (tb + 1) * TBW], pq[:])

                units.append(unit)
        return kt, qt, units

    # interleave phase A with hp0's q/k projections; front-load A so the PE
    # never outruns the hs DMA stream, and defer hp0's last three q-units
    # into the attention slot queue (ib N needs only q block N)
    kt_cur, qt_cur, bu = b_units(0)
    bu = list(bu)
    defer_q = [bu.pop() for _ in range(3)][::-1]
    for tb in range(S // 128):
        a_unit(tb)
        if tb >= 6 and tb % 2 == 0 and bu:
            bu.pop(0)()
    while bu:
        bu.pop(0)()

    # ---- Phase C: attention with interleaved next-pair projections ----
    valsT_sb = [
        vtpool.tile([128, S], FP16, tag=f"vt{hp}", name=f"vt{hp}") for hp in range(nHP)
    ]

    queue = []    # deferred projection units for the next head pair
    pending = []  # deferred normalization closures

    def emit_av(e, jc, hp, pv_pair):
        for par in range(2):
            nc.tensor.matmul(
                pv_pair[par][:],
                v_sb[jc][:, (2 * hp + par) * 65 : (2 * hp + par + 1) * 65],
                e[:, par * 512 : (par + 1) * 512],
                start=(jc == 0),
                stop=(jc == nJC - 1),
            )

    post = []     # deferred pv -> SBUF drains (run early next query block)

    def make_post(hp, ibs, pv_pair, par):
        h_rows = slice(par * 64, (par + 1) * 64)

        def drain():
            d = rpool.tile([1, IBW], F32, tag="d", name="d")
            nc.vector.tensor_copy(d[:], pv_pair[par][64:65, :])
            r = rpool.tile([1, IBW], F32, tag="r", name="r", bufs=4)
            nc.vector.reciprocal_approx_fast(out=r[:], in_=d[:])
            nc.vector.tensor_copy(valsT_sb[hp][h_rows, ibs], pv_pair[par][0:64, :])

            def norm():
                pb = prps.tile([128, TBW], F32, tag="pr", name="pb")[:, 0:IBW]
                nc.tensor.matmul(pb[:], ones32[:], r[:], start=True, stop=True)
                nc.vector.tensor_mul(
                    valsT_sb[hp][h_rows, ibs],
                    valsT_sb[hp][h_rows, ibs],
                    pb[h_rows, :],
                )

            pending.append(norm)

        return drain

    queue.extend(defer_q)
    for hp in range(nHP):
        nxt = b_units(hp + 1) if hp + 1 < nHP else None
        if nxt:
            queue.extend(nxt[2])

        for ib in range(nIB):
            ibs = slice(ib * IBW, (ib + 1) * IBW)
            pv_pair = [
                pvps.tile([128, IBW], F32, tag=f"pv{par}", name=f"pv{par}")[0:65, :]
                for par in range(2)
            ]
            pend = []
            for jc in range(nJC):
                sc = scps.tile([128, 1024], F32, tag="sc", name="sc")
                for par in range(2):
                    nc.tensor.matmul(
                        sc[:, par * 512 : (par + 1) * 512],
                        kt_cur[par * 64 : (par + 1) * 64, jc * 128 : (jc + 1) * 128],
                        qt_cur[par * 64 : (par + 1) * 64, ibs],
                        start=True,
                        stop=True,
                    )
                if len(pend) >= 2:
                    emit_av(*pend.pop(0), hp, pv_pair)
                # slot schedule: pack PE-heavy fillers into the pipeline-fill
                # phase (jc 0-3) so the HAM activity window never sees a lull
                if jc in (0, 3):
                    if queue:
                        queue.pop(0)()
                    else:
                        # HAM-warm filler on resident data (dependency-free so
                        # it can never head-of-line-block the PE); discarded
                        pw = prps.tile([128, TBW], F32, tag="pr", name="pw")
                        for _ in range(3):
                            nc.tensor.matmul(
                                pw[:], kt_cur[0:64, 0:128], qt_cur[0:64, 0:512],
                                start=True, stop=True,
                            )
                elif jc in (1, 2) and post:
                    post.pop(0)()
                elif jc in (5, 7) and pending:
                    pending.pop(0)()
                e = work.tile([128, 1024], FP16, tag="e")
                if jc % APPROX_PERIOD == 0:
                    nc.vector.tensor_scalar(
                        e[:].bitcast(I16)[:], sc[:], EXP_S, EXP_B, MULT, ADD
                    )
                else:
                    nc.scalar.activation(e[:], sc[:], EXP, scale=0.125)
                pend.append((e, jc))
            while pend:
                emit_av(*pend.pop(0), hp, pv_pair)
            for par in range(2):
                post.append(make_post(hp, ibs, pv_pair, par))
        if nxt:
            kt_cur, qt_cur = nxt[0], nxt[1]

    while queue:
        queue.pop(0)()
    while post:
        post.pop(0)()
    while pending:
        pending.pop(0)()

    # ---- Phase D: o-projection, PSUM-accumulated over head pairs.
    # tb pairs share each LDWEIGHTS of the stationary wo chunk.
    for mb in range(nF):
        for tbp in range(nTB // 2):
            pos = [prps.tile([128, TBW], F32, tag="pr", name=f"po{t}") for t in range(2)]
            for hp in range(nHP):
                for t in range(2):
                    nc.tensor.matmul(
                        pos[t][:],
                        wo_sb[hp][:, mb * 128 : (mb + 1) * 128],
                        valsT_sb[hp][:, (2 * tbp + t) * TBW : (2 * tbp + t + 1) * TBW],
                        start=(hp == 0),
                        stop=(hp == nHP - 1),
                    )
            for t in range(2):
                tb = 2 * tbp + t
                ot = ostage.tile([128, TBW], F32, tag="ot")
                nc.scalar.copy(ot[:], pos[t][:])
                nc.sync.dma_start(
                    out=io["outT"][mb * 128 : (mb + 1) * 128, tb * TBW : (tb + 1) * TBW],
                    in_=ot[:],
                )


# ---- host orchestration ----


import numpy as np

N_CORES = 8
B_FULL, S_FULL, F_FULL = 4, 2048, 1024
H_TOTAL = 16
H_PER_CORE = H_TOTAL // 2

_compiled = {}


def _get_compiled():
    if "nc" not in _compiled:
        from contextlib import ExitStack

        from concourse import bacc

        nc = bacc.Bacc(
            "TRN2", target_bir_lowering=False, debug=False, num_devices=N_CORES
        )
        io = declare_io(nc, S_FULL, F_FULL, H_PER_CORE)
        with tile.TileContext(nc) as tc:
            with ExitStack() as ctx:
                build(ctx, tc, io, S_FULL, F_FULL, H_PER_CORE)
        nc.compile()
        _compiled["nc"] = nc
    return _compiled["nc"]


def _shard_inputs(h_source, h_target, w_qk, w_v, w_o):
    """Per-core input maps. Core c -> batch c//2, head-half c%2."""

    def c16(x):
        return np.ascontiguousarray(x.astype(np.float16))

    in_maps = []
    for core in range(N_CORES):
        b, hh = divmod(core, 2)
        heads = range(hh * H_PER_CORE, (hh + 1) * H_PER_CORE)
        wq = np.concatenate([w_qk[h * 128 : h * 128 + 64] for h in heads], 0)
        wk = np.concatenate([w_qk[h * 128 + 64 : (h + 1) * 128] for h in heads], 0)
        wv = np.concatenate([w_v[h * 64 : (h + 1) * 64] for h in heads], 0)
        dcols = np.concatenate([np.arange(h * 64, (h + 1) * 64) for h in heads])
        in_maps.append(
            {
                "hsT": c16(h_source[b].T),
                "htT": c16(h_target[b].T),
                "wqT": c16(wq.T),
                "wkT": c16(wk.T),
                "wvT": c16(wv.T),
                "woT": c16(w_o[:, dcols].T),
            }
        )
    return in_maps


def _run(h_source, h_target, w_qk, w_v, w_o, b_o, trace=False, trace_cores=None):
    from concourse.bass_utils import run_bass_kernel_spmd

    nc = _get_compiled()
    in_maps = _shard_inputs(
        np.asarray(h_source, np.float32),
        np.asarray(h_target, np.float32),
        np.asarray(w_qk, np.float32),
        np.asarray(w_v, np.float32),
        np.asarray(w_o, np.float32),
    )
    res = run_bass_kernel_spmd(
        nc,
        in_maps,
        core_ids=list(range(N_CORES)),
        trace=trace,
        trace_cores=trace_cores,
    )
    b_o = np.asarray(b_o, np.float32)
    out = np.empty((B_FULL, S_FULL, F_FULL), np.float32)
    for b in range(B_FULL):
        acc = res.results[2 * b]["outT"] + res.results[2 * b + 1]["outT"]
        out[b] = acc.T + b_o
    return out, res


def kernel(h_source, h_target, w_qk, w_v, w_o, b_o):
    out, _ = _run(h_source, h_target, w_qk, w_v, w_o, b_o)
    return out
